# revision 10
# baseline (speedup 1.0000x reference)
"""Trainium2 Bass kernel for nn_MESNReadout (multi-layer echo state network readout).

Strategy
--------
Pure data parallelism over batch: B=512 -> 64 rows per core on 8 cores; all
weights replicated; output gathered on host.

The reference is a T=1024 sequential scan with L=3 stacked reservoir layers
plus a leaky-integrator side state xv. We reformulate with a *layer-skewed
wavefront*: wavefront k computes x0(k), x1(k-1), x2(k-2), hv(k-3)
simultaneously, where hv(t) = tanh(zv(t)) is the inner tanh of the xv
update. Every input a wavefront needs then comes from the previous
wavefront's tanh output T_{k-1} plus a staged history [x0(k-4); x1(k-4);
x2(k-4)] for the xv pooling term. One wavefront is:

  PE:  projA/projB (input projections, PSUM slot init, prefetched PF ahead)
       mm_b  (pool history -> zv rows, off critical path)
       mm_a  (recurrent matmul, the only op on the dependent chain)
  ACT: one tanh PSUM->SBUF
  DVE: three small history copies (a wavefront of slack)

The critical cycle is mm_a -> tanh -> mm_a: the minimal PE->ACT->PE round
trip this recurrence permits. State layout is transposed ([feature, batch])
so matmuls contract over partitions, and *padded* to partition-aligned
blocks x0@[0:20] x1@[32:52] x2@[64:84] hv@[96:108] because engines can only
address SBUF partition ranges starting at 0/32/64/96 and matmul outputs
must start at PSUM partition 0/32/64. Gap rows carry zeros (weights are
zero-padded). The host pre-packs u into a paired time-shifted array
up[128, T+5, 64] (rows 0:64 = uT(j-2), rows 64:128 = uT(j-3)) so one
projection matmul covers two skewed time blocks and boundary conditions
fall out as zeros.
"""
import sys

import numpy as np

sys.path.insert(0, "/opt/trn_rl_repo")

L, S, TH, D = 3, 4, 5, 64
NCLS = 100
B = 512
DELTA = 0.9
NCORES = 8
BC = B // NCORES            # 64 batch rows per core
R = L * S * TH              # 60
LS = L * S                  # 12
F = R + LS                  # 72 logical state rows
SS = 108                    # padded state span
NB = 6                      # rotating state/history buffers
NS = 8                      # rotating PSUM slots: one full bank each, because
                            # matmul start=True zeroes the entire 2KB bank
PF = 4                      # projection prefetch distance (slots ahead)
UCHUNK = 16                 # timesteps of `up` per DMA chunk
UAHEAD = 4                  # u chunks to stay ahead of consumption

# padded positions of the 72 logical rows [x0(20) x1(20) x2(20) hv(12)]
NEWPOS = np.concatenate([np.arange(0, 20), np.arange(32, 52),
                         np.arange(64, 84), np.arange(96, 108)])


def _bd(Ws):
    a, b = Ws.shape[1], Ws.shape[2]
    M = np.zeros((S * a, S * b), np.float32)
    for s in range(S):
        M[s * a:(s + 1) * a, s * b:(s + 1) * b] = Ws[s]
    return M


def _hstack_s(Ws):
    return np.concatenate([Ws[s] for s in range(S)], axis=1).astype(np.float32)


def build_host_mats(W_in0, W_in_rest, W, Wv_in, Wv, W_out):
    MpT = np.zeros((LS, R), np.float32)
    for d in range(L):
        for s in range(S):
            MpT[4 * d + s, 20 * d + 5 * s:20 * d + 5 * s + TH] = 1.0 / TH

    # compact [72,72] recurrent matrix in logical order [x0 x1 x2 hv]
    Wc = np.zeros((F, F), np.float32)
    Wc[0:20, 0:20] = _bd(W[0])
    Wc[0:20, 20:40] = _bd(W_in_rest[0][:, D:, :])
    Wc[20:40, 20:40] = _bd(W[1])
    Wc[20:40, 40:60] = _bd(W_in_rest[1][:, D:, :])
    Wc[40:60, 40:60] = _bd(W[2])
    Wc[60:72, 60:72] = DELTA * Wv.T
    BigWa = np.zeros((SS, SS), np.float32)
    BigWa[np.ix_(NEWPOS, NEWPOS)] = Wc

    # input projections: WA -> out rows [0:52] = [U0 | gap | U1],
    # WB -> out rows [64:108] = [U2 | gap | Uv]
    WA = np.zeros((128, 52), np.float32)
    WA[0:64, 0:20] = _hstack_s(W_in0)
    WA[64:128, 32:52] = _hstack_s(W_in_rest[0][:, :D, :])
    WB = np.zeros((128, 44), np.float32)
    WB[0:64, 0:20] = _hstack_s(W_in_rest[1][:, :D, :])
    WB[64:128, 32:44] = Wv_in.T.astype(np.float32)

    # pool-history -> zv: out rows [64:108], cols 32:44 live
    Gw = ((1.0 - DELTA) * (Wv @ MpT)).T.astype(np.float32)   # [60, 12]
    Gwp = np.zeros((96, 44), np.float32)
    Gwp[0:20, 32:44] = Gw[0:20]
    Gwp[32:52, 32:44] = Gw[20:40]
    Gwp[64:84, 32:44] = Gw[40:60]

    # xv(T-1) = 0.1*pool(x(T-1)) + 0.9*hv(T-1) over padded feats rows
    poolhv = np.zeros((SS, LS), np.float32)
    poolhv[NEWPOS[0:60], :] = (1.0 - DELTA) * MpT.T
    poolhv[96:108, :] = DELTA * np.eye(LS, dtype=np.float32)

    woutp = np.zeros((SS, NCLS), np.float32)
    woutp[NEWPOS, :] = W_out.astype(np.float32)
    return BigWa, Gwp, WA, WB, poolhv, woutp


def build_up(u_core, T):
    """u_core [BC, T, 64] -> up [128, T+5, BC] f32 (paired, shifted, padded)."""
    uT = np.ascontiguousarray(u_core.transpose(2, 1, 0)).astype(np.float32)
    up = np.zeros((128, T + 5, u_core.shape[0]), np.float32)
    up[0:64, 2:T + 2] = uT
    up[64:128, 3:T + 3] = uT
    return np.ascontiguousarray(up)


def build_nc(T):
    import concourse.bacc as bacc
    import concourse.mybir as mybir
    from concourse.tile import TileContext

    dt = mybir.dt.float32
    NW = T + 3
    NUP = T + 5
    n_chunks = (NUP + UCHUNK - 1) // UCHUNK

    nc = bacc.Bacc(None)
    up_d = nc.dram_tensor("up", [128, NUP, BC], dt, kind="ExternalInput")
    bigwa_d = nc.dram_tensor("bigwa", [SS, SS], dt, kind="ExternalInput")
    gw_d = nc.dram_tensor("gw", [96, 44], dt, kind="ExternalInput")
    wa_d = nc.dram_tensor("wa", [128, 52], dt, kind="ExternalInput")
    wb_d = nc.dram_tensor("wb", [128, 44], dt, kind="ExternalInput")
    poolhv_d = nc.dram_tensor("poolhv", [SS, LS], dt, kind="ExternalInput")
    wout_d = nc.dram_tensor("wout", [SS, NCLS], dt, kind="ExternalInput")
    bout_d = nc.dram_tensor("bout", [NCLS, 1], dt, kind="ExternalInput")
    out_d = nc.dram_tensor("out", [NCLS, BC], dt, kind="ExternalOutput")

    with TileContext(nc) as tc:
        with (
            tc.tile_pool(name="const", bufs=1) as cpool,
            tc.tile_pool(name="ubuf", bufs=6) as upool,
            tc.tile_pool(name="state", bufs=1) as spool,
            tc.tile_pool(name="psum", bufs=1, space="PSUM") as ppool,
        ):
            bigwa = cpool.tile([SS, SS], dt)
            gw = cpool.tile([96, 44], dt)
            wa = cpool.tile([128, 52], dt)
            wb = cpool.tile([128, 44], dt)
            poolhv = cpool.tile([SS, LS], dt)
            wout = cpool.tile([SS, NCLS], dt)
            bout = cpool.tile([NCLS, 1], dt)
            for sb, dr in ((bigwa, bigwa_d), (gw, gw_d), (wa, wa_d),
                           (wb, wb_d), (poolhv, poolhv_d),
                           (wout, wout_d), (bout, bout_d)):
                nc.sync.dma_start(sb[:], dr[:])

            # rb[:, j%NB, :] = T_{j-1} (tanh output of wavefront j-1), padded
            rb = spool.tile([SS, NB, BC], dt)
            # hist[:, j%NB, :] = [x0(j-4) | gap | x1(j-4) | gap | x2(j-4)]
            hist = spool.tile([96, NB, BC], dt)
            nc.vector.memset(rb[:], 0.0)
            nc.vector.memset(hist[:], 0.0)

            # one PSUM region: slot j = one full 2KB bank, cols 0:BC used
            psum = ppool.tile([128, NS, 512], dt)
            nc.vector.memset(psum[:], 0.0)

            u_tiles = [None] * n_chunks
            dma_eng = [nc.sync, nc.gpsimd, nc.scalar]

            def load_chunk(c):
                if c >= n_chunks or u_tiles[c] is not None:
                    return
                j0 = c * UCHUNK
                w = min(UCHUNK, NUP - j0)
                t = upool.tile([128, UCHUNK, BC], dt, tag="uc")
                dma_eng[c % len(dma_eng)].dma_start(
                    t[:, :w, :], up_d[:, j0:j0 + w, :])
                u_tiles[c] = t

            def up_ap(j):
                c, o = divmod(j, UCHUNK)
                return u_tiles[c][:, o, :]

            def emit_proj(k):
                if k >= NW:
                    return
                sl = psum[:, k % NS, 0:BC]
                nc.tensor.matmul(sl[0:52, :], wa[:], up_ap(k + 2),
                                 start=True, stop=False, skip_group_check=True)
                nc.tensor.matmul(sl[64:108, :], wb[:], up_ap(k),
                                 start=True, stop=False, skip_group_check=True)

            for c in range(UAHEAD):
                load_chunk(c)
            for k in range(PF):
                emit_proj(k)

            for k in range(NW):
                if k % UCHUNK == 0:
                    load_chunk(k // UCHUNK + UAHEAD)
                emit_proj(k + PF)
                sl = psum[:, k % NS, 0:BC]
                # xv pooling term from staged history (off critical path)
                nc.tensor.matmul(sl[64:108, :], gw[:], hist[:, k % NB, :],
                                 start=False, stop=False, skip_group_check=True)
                # the recurrent matmul: the only op on the dependent chain
                nc.tensor.matmul(sl[0:SS, :], bigwa[:], rb[:, k % NB, :],
                                 start=False, stop=True, skip_group_check=True)
                nc.scalar.activation(rb[:, (k + 1) % NB, :], sl[0:SS, :],
                                     mybir.ActivationFunctionType.Tanh)
                # stage history for wavefront k+1: x(k-3)
                if k + 1 < NW:
                    nc.vector.tensor_copy(hist[0:20, (k + 1) % NB, :],
                                          rb[0:20, (k - 2) % NB, :])
                    nc.vector.tensor_copy(hist[32:52, (k + 1) % NB, :],
                                          rb[32:52, (k - 1) % NB, :])
                    nc.vector.tensor_copy(hist[64:84, (k + 1) % NB, :],
                                          rb[64:84, k % NB, :])

            # ---- tail: feats = [x0|x1|x2|xv](T-1) padded, then readout ----
            feats = spool.tile([SS, BC], dt)
            nc.vector.memset(feats[:], 0.0)
            nc.vector.tensor_copy(feats[0:20, :], rb[0:20, T % NB, :])
            nc.vector.tensor_copy(feats[32:52, :], rb[32:52, (T + 1) % NB, :])
            nc.vector.tensor_copy(feats[64:84, :], rb[64:84, (T + 2) % NB, :])
            nc.vector.tensor_copy(feats[96:108, :], rb[96:108, (T + 3) % NB, :])
            nc.tensor.matmul(psum[0:LS, 0, 0:BC], poolhv[:], feats[0:SS, :],
                             start=True, stop=True, skip_group_check=True)
            nc.vector.tensor_copy(feats[96:108, :], psum[0:LS, 0, 0:BC])
            nc.tensor.matmul(psum[0:NCLS, 1, 0:BC], wout[:], feats[0:SS, :],
                             start=True, stop=True, skip_group_check=True)
            out_sb = spool.tile([NCLS, BC], dt)
            nc.scalar.activation(out_sb[:], psum[0:NCLS, 1, 0:BC],
                                 mybir.ActivationFunctionType.Identity,
                                 bias=bout[:, 0:1])
            nc.sync.dma_start(out_d[:], out_sb[:])

    nc.compile()
    return nc


_NC_CACHE = {}


def _get_nc(T):
    if T not in _NC_CACHE:
        _NC_CACHE[T] = build_nc(T)
    return _NC_CACHE[T]


def kernel(u, W_in0, W_in_rest, W, Wv_in, Wv, W_out, b_out,
           _T=None, _trace=False):
    from concourse.bass_utils import run_bass_kernel_spmd

    u = np.asarray(u, np.float32)
    T = _T or u.shape[1]
    BigWa, Gwp, WA, WB, poolhv, woutp = build_host_mats(
        np.asarray(W_in0, np.float32), np.asarray(W_in_rest, np.float32),
        np.asarray(W, np.float32), np.asarray(Wv_in, np.float32),
        np.asarray(Wv, np.float32), np.asarray(W_out, np.float32))
    bout = np.ascontiguousarray(
        np.asarray(b_out, np.float32).reshape(NCLS, 1))

    nc = _get_nc(T)
    in_maps = []
    for c in range(NCORES):
        in_maps.append({
            "up": build_up(u[c * BC:(c + 1) * BC, :T, :], T),
            "bigwa": BigWa, "gw": Gwp, "wa": WA, "wb": WB,
            "poolhv": poolhv, "wout": woutp, "bout": bout,
        })
    res = run_bass_kernel_spmd(nc, in_maps, core_ids=list(range(NCORES)),
                               trace=_trace)
    outs = [res.results[c]["out"] for c in range(NCORES)]
    full = np.concatenate([np.asarray(o).T for o in outs], axis=0)
    kernel.last_results = res
    return full.astype(np.float32)


# revision 11
# speedup vs baseline: 83.4832x; 83.4832x over previous
"""Trainium2 Bass kernel for nn_MESNReadout (multi-layer echo state network readout).

Strategy
--------
Pure data parallelism over batch: B=512 -> 64 rows per core on 8 cores; all
weights replicated; output gathered on host.

The reference is a T=1024 sequential scan with L=3 stacked reservoir layers
plus a leaky-integrator side state xv. We reformulate with a *layer-skewed
wavefront*: wavefront k computes x0(k), x1(k-1), x2(k-2), hv(k-3)
simultaneously, where hv(t) = tanh(zv(t)) is the inner tanh of the xv
update. Every input a wavefront needs then comes from the previous
wavefront's tanh output T_{k-1} plus a staged history [x0(k-4); x1(k-4);
x2(k-4)] for the xv pooling term. One wavefront is:

  PE:  projA/projB (input projections, PSUM slot init, prefetched PF ahead)
       mm_b  (pool history -> zv rows, off critical path)
       mm_a  (recurrent matmul, the only op on the dependent chain)
  ACT: one tanh PSUM->SBUF
  DVE: three small history copies (a wavefront of slack)

The critical cycle is mm_a -> tanh -> mm_a: the minimal PE->ACT->PE round
trip this recurrence permits. State layout is transposed ([feature, batch])
so matmuls contract over partitions, and *padded* to partition-aligned
blocks x0@[0:20] x1@[32:52] x2@[64:84] hv@[96:108] because engines can only
address SBUF partition ranges starting at 0/32/64/96 and matmul outputs
must start at PSUM partition 0/32/64. Gap rows carry zeros (weights are
zero-padded). The host pre-packs u into a paired time-shifted array
up[128, T+5, 64] (rows 0:64 = uT(j-2), rows 64:128 = uT(j-3)) so one
projection matmul covers two skewed time blocks and boundary conditions
fall out as zeros.
"""
import sys

import numpy as np

sys.path.insert(0, "/opt/trn_rl_repo")

L, S, TH, D = 3, 4, 5, 64
NCLS = 100
B = 512
DELTA = 0.9
NCORES = 8
BC = B // NCORES            # 64 batch rows per core
R = L * S * TH              # 60
LS = L * S                  # 12
F = R + LS                  # 72 logical state rows
SS = 108                    # padded state span
NB = 6                      # rotating state/history buffers
NS = 8                      # rotating PSUM slots: one full bank each, because
                            # matmul start=True zeroes the entire 2KB bank
PF = 4                      # projection prefetch distance (slots ahead)
UCHUNK = 16                 # timesteps of `up` per DMA chunk
UAHEAD = 4                  # u chunks to stay ahead of consumption

# padded positions of the 72 logical rows [x0(20) x1(20) x2(20) hv(12)]
NEWPOS = np.concatenate([np.arange(0, 20), np.arange(32, 52),
                         np.arange(64, 84), np.arange(96, 108)])


def _bd(Ws):
    a, b = Ws.shape[1], Ws.shape[2]
    M = np.zeros((S * a, S * b), np.float32)
    for s in range(S):
        M[s * a:(s + 1) * a, s * b:(s + 1) * b] = Ws[s]
    return M


def _hstack_s(Ws):
    return np.concatenate([Ws[s] for s in range(S)], axis=1).astype(np.float32)


def build_host_mats(W_in0, W_in_rest, W, Wv_in, Wv, W_out):
    MpT = np.zeros((LS, R), np.float32)
    for d in range(L):
        for s in range(S):
            MpT[4 * d + s, 20 * d + 5 * s:20 * d + 5 * s + TH] = 1.0 / TH

    # compact [72,72] recurrent matrix in logical order [x0 x1 x2 hv]
    Wc = np.zeros((F, F), np.float32)
    Wc[0:20, 0:20] = _bd(W[0])
    Wc[0:20, 20:40] = _bd(W_in_rest[0][:, D:, :])
    Wc[20:40, 20:40] = _bd(W[1])
    Wc[20:40, 40:60] = _bd(W_in_rest[1][:, D:, :])
    Wc[40:60, 40:60] = _bd(W[2])
    Wc[60:72, 60:72] = DELTA * Wv.T
    BigWa = np.zeros((SS, SS), np.float32)
    BigWa[np.ix_(NEWPOS, NEWPOS)] = Wc

    # input projections: WA -> out rows [0:52] = [U0 | gap | U1],
    # WB -> out rows [64:108] = [U2 | gap | Uv]
    WA = np.zeros((128, 52), np.float32)
    WA[0:64, 0:20] = _hstack_s(W_in0)
    WA[64:128, 32:52] = _hstack_s(W_in_rest[0][:, :D, :])
    WB = np.zeros((128, 44), np.float32)
    WB[0:64, 0:20] = _hstack_s(W_in_rest[1][:, :D, :])
    WB[64:128, 32:44] = Wv_in.T.astype(np.float32)

    # pool-history -> zv: out rows [64:108], cols 32:44 live
    Gw = ((1.0 - DELTA) * (Wv @ MpT)).T.astype(np.float32)   # [60, 12]
    Gwp = np.zeros((96, 44), np.float32)
    Gwp[0:20, 32:44] = Gw[0:20]
    Gwp[32:52, 32:44] = Gw[20:40]
    Gwp[64:84, 32:44] = Gw[40:60]

    # xv(T-1) = 0.1*pool(x(T-1)) + 0.9*hv(T-1) over padded feats rows
    poolhv = np.zeros((SS, LS), np.float32)
    poolhv[NEWPOS[0:60], :] = (1.0 - DELTA) * MpT.T
    poolhv[96:108, :] = DELTA * np.eye(LS, dtype=np.float32)

    woutp = np.zeros((SS, NCLS), np.float32)
    woutp[NEWPOS, :] = W_out.astype(np.float32)
    return BigWa, Gwp, WA, WB, poolhv, woutp


def build_up(u_core, T):
    """u_core [BC, T, 64] -> up [128, T+5, BC] f32 (paired, shifted, padded)."""
    uT = np.ascontiguousarray(u_core.transpose(2, 1, 0)).astype(np.float32)
    up = np.zeros((128, T + 5, u_core.shape[0]), np.float32)
    up[0:64, 2:T + 2] = uT
    up[64:128, 3:T + 3] = uT
    return np.ascontiguousarray(up)


def build_nc(T, prec="f32"):
    import concourse.bacc as bacc
    import concourse.mybir as mybir
    from concourse.tile import TileContext

    dt = mybir.dt.float32
    dtb = mybir.dt.bfloat16 if prec in ("bf16", "bf16all") else mybir.dt.float32
    dtu = mybir.dt.bfloat16 if prec == "bf16all" else mybir.dt.float32
    NW = T + 3
    NUP = T + 5
    n_chunks = (NUP + UCHUNK - 1) // UCHUNK

    nc = bacc.Bacc(None)
    up_d = nc.dram_tensor("up", [128, NUP, BC], dtu, kind="ExternalInput")
    bigwa_d = nc.dram_tensor("bigwa", [SS, SS], dtb, kind="ExternalInput")
    gw_d = nc.dram_tensor("gw", [96, 44], dtb, kind="ExternalInput")
    wa_d = nc.dram_tensor("wa", [128, 52], dtu, kind="ExternalInput")
    wb_d = nc.dram_tensor("wb", [128, 44], dtu, kind="ExternalInput")
    poolhv_d = nc.dram_tensor("poolhv", [SS, LS], dt, kind="ExternalInput")
    wout_d = nc.dram_tensor("wout", [SS, NCLS], dt, kind="ExternalInput")
    bout_d = nc.dram_tensor("bout", [NCLS, 1], dt, kind="ExternalInput")
    out_d = nc.dram_tensor("out", [NCLS, BC], dt, kind="ExternalOutput")

    with TileContext(nc) as tc:
        with (
            tc.tile_pool(name="const", bufs=1) as cpool,
            tc.tile_pool(name="ubuf", bufs=6) as upool,
            tc.tile_pool(name="state", bufs=1) as spool,
            tc.tile_pool(name="psum", bufs=1, space="PSUM") as ppool,
        ):
            bigwa = cpool.tile([SS, SS], dtb)
            gw = cpool.tile([96, 44], dtb)
            wa = cpool.tile([128, 52], dtu)
            wb = cpool.tile([128, 44], dtu)
            poolhv = cpool.tile([SS, LS], dt)
            wout = cpool.tile([SS, NCLS], dt)
            bout = cpool.tile([NCLS, 1], dt)
            for sb, dr in ((bigwa, bigwa_d), (gw, gw_d), (wa, wa_d),
                           (wb, wb_d), (poolhv, poolhv_d),
                           (wout, wout_d), (bout, bout_d)):
                nc.sync.dma_start(sb[:], dr[:])

            # rb[:, j%NB, :] = T_{j-1} (tanh output of wavefront j-1), padded
            rb = spool.tile([SS, NB, BC], dtb)
            # hist[:, j%NB, :] = [x0(j-4) | gap | x1(j-4) | gap | x2(j-4)]
            hist = spool.tile([96, NB, BC], dtb)
            nc.vector.memset(rb[:], 0.0)
            nc.vector.memset(hist[:], 0.0)

            # one PSUM region: slot j = one full 2KB bank, cols 0:BC used
            psum = ppool.tile([128, NS, 512], dt)
            nc.vector.memset(psum[:], 0.0)

            u_tiles = [None] * n_chunks
            dma_eng = [nc.sync, nc.gpsimd, nc.scalar]

            def load_chunk(c):
                if c >= n_chunks or u_tiles[c] is not None:
                    return
                j0 = c * UCHUNK
                w = min(UCHUNK, NUP - j0)
                t = upool.tile([128, UCHUNK, BC], dtu, tag="uc")
                dma_eng[c % len(dma_eng)].dma_start(
                    t[:, :w, :], up_d[:, j0:j0 + w, :])
                u_tiles[c] = t

            def up_ap(j):
                c, o = divmod(j, UCHUNK)
                return u_tiles[c][:, o, :]

            def emit_proj(k):
                if k >= NW:
                    return
                sl = psum[:, k % NS, 0:BC]
                nc.tensor.matmul(sl[0:52, :], wa[:], up_ap(k + 2),
                                 start=True, stop=False, skip_group_check=True)
                nc.tensor.matmul(sl[64:108, :], wb[:], up_ap(k),
                                 start=True, stop=False, skip_group_check=True)

            for c in range(UAHEAD):
                load_chunk(c)
            for k in range(PF):
                emit_proj(k)

            for k in range(NW):
                if k % UCHUNK == 0:
                    load_chunk(k // UCHUNK + UAHEAD)
                emit_proj(k + PF)
                sl = psum[:, k % NS, 0:BC]
                # xv pooling term from staged history (off critical path)
                nc.tensor.matmul(sl[64:108, :], gw[:], hist[:, k % NB, :],
                                 start=False, stop=False, skip_group_check=True)
                # the recurrent matmul: the only op on the dependent chain
                nc.tensor.matmul(sl[0:SS, :], bigwa[:], rb[:, k % NB, :],
                                 start=False, stop=True, skip_group_check=True)
                nc.scalar.activation(rb[:, (k + 1) % NB, :], sl[0:SS, :],
                                     mybir.ActivationFunctionType.Tanh)
                # stage history for wavefront k+1: x(k-3)
                if k + 1 < NW:
                    nc.vector.tensor_copy(hist[0:20, (k + 1) % NB, :],
                                          rb[0:20, (k - 2) % NB, :])
                    nc.vector.tensor_copy(hist[32:52, (k + 1) % NB, :],
                                          rb[32:52, (k - 1) % NB, :])
                    nc.vector.tensor_copy(hist[64:84, (k + 1) % NB, :],
                                          rb[64:84, k % NB, :])

            # ---- tail: feats = [x0|x1|x2|xv](T-1) padded, then readout ----
            feats = spool.tile([SS, BC], dt)
            nc.vector.memset(feats[:], 0.0)
            nc.vector.tensor_copy(feats[0:20, :], rb[0:20, T % NB, :])
            nc.vector.tensor_copy(feats[32:52, :], rb[32:52, (T + 1) % NB, :])
            nc.vector.tensor_copy(feats[64:84, :], rb[64:84, (T + 2) % NB, :])
            nc.vector.tensor_copy(feats[96:108, :], rb[96:108, (T + 3) % NB, :])
            nc.tensor.matmul(psum[0:LS, 0, 0:BC], poolhv[:], feats[0:SS, :],
                             start=True, stop=True, skip_group_check=True)
            nc.vector.tensor_copy(feats[96:108, :], psum[0:LS, 0, 0:BC])
            nc.tensor.matmul(psum[0:NCLS, 1, 0:BC], wout[:], feats[0:SS, :],
                             start=True, stop=True, skip_group_check=True)
            out_sb = spool.tile([NCLS, BC], dt)
            nc.scalar.activation(out_sb[:], psum[0:NCLS, 1, 0:BC],
                                 mybir.ActivationFunctionType.Identity,
                                 bias=bout[:, 0:1])
            nc.sync.dma_start(out_d[:], out_sb[:])

    nc.compile()
    return nc


_NC_CACHE = {}


def _get_nc(T, prec="f32"):
    key = (T, prec)
    if key not in _NC_CACHE:
        _NC_CACHE[key] = build_nc(T, prec)
    return _NC_CACHE[key]


def kernel(u, W_in0, W_in_rest, W, Wv_in, Wv, W_out, b_out,
           _T=None, _trace=False, _prec="f32"):
    from concourse.bass_utils import run_bass_kernel_spmd
    import ml_dtypes

    u = np.asarray(u, np.float32)
    T = _T or u.shape[1]
    cb = (lambda x: np.ascontiguousarray(x.astype(ml_dtypes.bfloat16))) \
        if _prec in ("bf16", "bf16all") else (lambda x: x)
    cu = (lambda x: np.ascontiguousarray(x.astype(ml_dtypes.bfloat16))) \
        if _prec == "bf16all" else (lambda x: x)
    BigWa, Gwp, WA, WB, poolhv, woutp = build_host_mats(
        np.asarray(W_in0, np.float32), np.asarray(W_in_rest, np.float32),
        np.asarray(W, np.float32), np.asarray(Wv_in, np.float32),
        np.asarray(Wv, np.float32), np.asarray(W_out, np.float32))
    bout = np.ascontiguousarray(
        np.asarray(b_out, np.float32).reshape(NCLS, 1))

    nc = _get_nc(T, _prec)
    in_maps = []
    for c in range(NCORES):
        in_maps.append({
            "up": cu(build_up(u[c * BC:(c + 1) * BC, :T, :], T)),
            "bigwa": cb(BigWa), "gw": cb(Gwp), "wa": cu(WA), "wb": cu(WB),
            "poolhv": poolhv, "wout": woutp, "bout": bout,
        })
    res = run_bass_kernel_spmd(nc, in_maps, core_ids=list(range(NCORES)),
                               trace=_trace)
    outs = [res.results[c]["out"] for c in range(NCORES)]
    full = np.concatenate([np.asarray(o).T for o in outs], axis=0)
    kernel.last_results = res
    return full.astype(np.float32)
